# revision 26
# baseline (speedup 1.0000x reference)
"""nn_AttentionBlock TRN2 kernel: 8-way batch-parallel across 8 NeuronCores.

Each core runs an identical Bass/Tile program over one image of the batch
(B=8): instance-norm -> q/k/v projections -> full 4096x4096 attention ->
output projection + residual. No collectives are needed; inputs are
sharded over batch and the weights are replicated to every core.

Per-core program (T=4096 tokens, C=512 channels; qkv/proj matmuls bf16,
attention matmuls fp8-e4m3 DoubleRow, fp32 PSUM accumulation everywhere):
  - x tiles are transposed on the (otherwise idle) TensorEngine via
    identity matmuls into PSUM, then cast to xT [C, T] bf16 on ScalarE.
    bn_stats runs on the fp32 PSUM transposes to get per-channel stats.
  - The instance-norm affine (xn = x*scale + shift, per channel) is FOLDED
    into the q/k/v weights and biases: W' = diag(scale) @ W (per-partition
    row scaling), bias' = b + shift @ W. xT never needs normalizing.
  - qT/kT [C, T] via matmuls with W' stationary; v [T, C] natural.
  - scores computed transposed [j, i] = kT.T @ qT / sqrt(C); exp on ScalarE
    straight out of PSUM (no max subtraction -- |scores| < ~2 here).
  - attn @ v accumulated over j in PSUM [c, i]; a parallel ones-row matmul
    accumulates the softmax denominator; normalization is applied after the
    final Wp matmul by row scaling (linearity).
  - out = xn + proj + bp, with xn recomputed in fp32 from a re-read of x.
"""

import hashlib
import os
import sys

sys.path.insert(0, "/opt/trn_rl_repo")

from contextlib import ExitStack

import numpy as np

import concourse.bass as bass
import concourse.bacc as bacc
import concourse.tile as tile
from concourse import mybir
from concourse.masks import make_identity

F32 = mybir.dt.float32
BF16 = mybir.dt.bfloat16
FP8 = mybir.dt.float8e4
EPS = 1e-3
P = 128

B, H, W, C = 8, 64, 64, 512
T = H * W
N_CORES = 8

IN_NAMES = ("x", "gamma", "beta", "Wq", "bq", "Wk", "bk", "Wv", "bv", "Wp", "bp")


def build_attn_tile(tc, outs, ins, T=T, C=C):
    """Emit the per-core attention-block program into TileContext `tc`.

    outs: {"out": AP [T, C] f32}; ins: dict of APs keyed like setup_inputs().
    """
    nc = tc.nc
    x = ins["x"]
    out = outs["out"]
    NCI = C // P          # channel chunks of 128
    NT = T // P           # token blocks of 128
    NI = T // 512         # i-chunks of 512 tokens
    NJ = NT               # j-blocks of 128 tokens
    SCALE = float(C) ** -0.5
    DR = mybir.MatmulPerfMode.DoubleRow

    ctx = ExitStack()
    with ctx:
        consts = ctx.enter_context(tc.tile_pool(name="consts", bufs=1))
        dram = ctx.enter_context(tc.tile_pool(name="dram", bufs=1, space="DRAM"))

        # ---- tiny constants needed by phase 1 -----------------------------
        eps_col = consts.tile([P, 1], F32)
        nc.vector.memset(eps_col, EPS)
        ident = consts.tile([P, P], F32)
        make_identity(nc, ident)
        # all-ones fp8 stationary for the softmax-denominator matmul; with
        # M=128 every output partition gets the same column sum, so the
        # denominator lands pre-broadcast [128, 512] in PSUM.
        ones8 = consts.tile([P, 2, P], FP8)
        nc.vector.memset(ones8, 1.0)

        # phase-scoped pools: freed before the attention phase opens
        qkv_ctx = ExitStack()
        wqkv = qkv_ctx.enter_context(tc.tile_pool(name="wqkv", bufs=1))
        wstage = qkv_ctx.enter_context(tc.tile_pool(name="wstage", bufs=1))
        stat = qkv_ctx.enter_context(tc.tile_pool(name="stat", bufs=2))

        # ---- phase 1 FIRST (so its DMAs win the queue): load x tiles,
        # transpose on the TensorEngine, cast to xT bf16, bn_stats on the
        # fp32 PSUM transposes.
        xT = wqkv.tile([P, NCI, T], BF16, tag="xT")
        NG = NT // 4  # token-tile groups of 4 (one PSUM bank per channel chunk)
        st_all = stat.tile([P, NCI, NG, 6], F32, tag="st_all")
        with (
            tc.tile_pool(name="xin", bufs=3) as xin,
            tc.tile_pool(name="pst", bufs=2, space="PSUM") as pst,
        ):
            for g in range(NG):
                pts = [pst.tile([P, 512], F32, tag=f"pt{ci}", name=f"pt{ci}_{g}") for ci in range(NCI)]
                # one 1 MiB DMA per group of 4 token tiles (row = a*128 + p)
                xt = xin.tile([P, 4, C], F32, tag="xt")
                nc.sync.dma_start(
                    out=xt,
                    in_=x[g * 512 : (g + 1) * 512, :].rearrange("(a p) c -> p a c", p=P),
                )
                for s in range(4):
                    for ci in range(NCI):
                        nc.tensor.transpose(
                            pts[ci][:, s * P : (s + 1) * P],
                            xt[:, s, ci * P : (ci + 1) * P],
                            ident,
                        )
                for ci in range(NCI):
                    nc.scalar.copy(out=xT[:, ci, g * 512 : (g + 1) * 512], in_=pts[ci])
                    nc.vector.bn_stats(out=st_all[:, ci, g, :], in_=pts[ci])

        # ---- per-channel affine params / weights (off the critical path) --
        gam_col = consts.tile([P, NCI], F32)
        bet_col = consts.tile([P, NCI], F32)
        bq_col = consts.tile([P, NCI], F32)
        bk_col = consts.tile([P, NCI], F32)
        bp_col = consts.tile([P, NCI], F32)
        for ci in range(NCI):
            sl = slice(ci * P, (ci + 1) * P)
            nc.sync.dma_start(out=gam_col[:, ci : ci + 1], in_=ins["gamma"][sl])
            nc.sync.dma_start(out=bet_col[:, ci : ci + 1], in_=ins["beta"][sl])
            nc.sync.dma_start(out=bq_col[:, ci : ci + 1], in_=ins["bq"][sl])
            nc.sync.dma_start(out=bk_col[:, ci : ci + 1], in_=ins["bk"][sl])
            nc.sync.dma_start(out=bp_col[:, ci : ci + 1], in_=ins["bp"][sl])
        bv_row = consts.tile([1, C], F32)
        nc.sync.dma_start(out=bv_row, in_=ins["bv"].rearrange("(o c) -> o c", o=1))

        # stage Wq/Wk/Wv in f32 (scaled by the norm below); Wp casts now
        w_stage = {}
        for wname in ("Wq", "Wk", "Wv"):
            st = wstage.tile([P, NCI, C], F32, tag=f"stage_{wname}", name=f"stage_{wname}")
            w_stage[wname] = st
            for ci in range(NCI):
                nc.sync.dma_start(out=st[:, ci, :], in_=ins[wname][ci * P : (ci + 1) * P, :])
        w_bf = {}
        for wname in ("Wq", "Wk", "Wv"):
            w_bf[wname] = wqkv.tile([P, NCI, C], BF16, tag=f"w_{wname}", name=f"w_{wname}")
        w_bf["Wp"] = consts.tile([P, NCI, C], BF16, tag="w_Wp", name="w_Wp")
        with tc.tile_pool(name="wpstage", bufs=2) as wps:
            for ci in range(NCI):
                st = wps.tile([P, C], F32, tag="wps")
                nc.sync.dma_start(out=st, in_=ins["Wp"][ci * P : (ci + 1) * P, :])
                nc.vector.tensor_copy(out=w_bf["Wp"][:, ci, :], in_=st)

        # ---- stats -> per-channel scale/shift; fold into weights ---------
        scl_sh_d = dram.tile([3, C], F32)  # rows: scale, shift+bp, bv'
        scale_cols = consts.tile([P, NCI], F32)
        shift_bf = consts.tile([P, NCI], BF16)
        for ci in range(NCI):
            mv = stat.tile([P, 2], F32, tag="bnmv")
            nc.vector.bn_aggr(out=mv, in_=st_all[:, ci])
            mean = mv[:, 0:1]
            var = mv[:, 1:2]
            # istd = 1/sqrt(var + eps), one Newton step on the inverse sqrt
            # (the ACT Sqrt spline has a loose precision budget).
            va = stat.tile([P, 1], F32, tag="va")
            nc.vector.tensor_scalar_add(out=va, in0=var, scalar1=EPS)
            s0 = stat.tile([P, 1], F32, tag="s0")
            nc.scalar.activation(
                out=s0, in_=var, func=mybir.ActivationFunctionType.Sqrt,
                bias=eps_col, scale=1.0,
            )
            r0 = stat.tile([P, 1], F32, tag="r0")
            nc.vector.reciprocal(out=r0, in_=s0)
            t0 = stat.tile([P, 1], F32, tag="t0")
            nc.vector.tensor_mul(out=t0, in0=r0, in1=r0)
            nc.vector.tensor_mul(out=t0, in0=t0, in1=va)
            nc.vector.tensor_scalar(
                out=t0, in0=t0, scalar1=-0.5, scalar2=1.5,
                op0=mybir.AluOpType.mult, op1=mybir.AluOpType.add,
            )
            istd = stat.tile([P, 1], F32, tag="istd")
            nc.vector.tensor_mul(out=istd, in0=r0, in1=t0)

            sc = scale_cols[:, ci : ci + 1]
            nc.vector.tensor_mul(out=sc, in0=gam_col[:, ci : ci + 1], in1=istd)
            shift_c = stat.tile([P, 1], F32, tag="shift_c")
            nc.vector.tensor_mul(out=shift_c, in0=mean, in1=sc)
            nc.vector.tensor_sub(out=shift_c, in0=bet_col[:, ci : ci + 1], in1=shift_c)
            nc.vector.tensor_copy(out=shift_bf[:, ci : ci + 1], in_=shift_c)
            sh2 = stat.tile([P, 1], F32, tag="sh2")
            nc.vector.tensor_add(out=sh2, in0=shift_c, in1=bp_col[:, ci : ci + 1])
            nc.sync.dma_start(out=scl_sh_d[0, ci * P : (ci + 1) * P], in_=sc)
            nc.sync.dma_start(out=scl_sh_d[1, ci * P : (ci + 1) * P], in_=sh2)

        # W' = diag(scale) @ W for q/k/v (per-partition row scale), bf16
        for wname in ("Wq", "Wk", "Wv"):
            st = w_stage[wname]
            for ci in range(NCI):
                nc.vector.tensor_scalar_mul(
                    out=w_bf[wname][:, ci, :], in0=st[:, ci, :],
                    scalar1=scale_cols[:, ci : ci + 1],
                )

        # folded biases: bias' = b + shift @ W'
        qbias = consts.tile([P, NCI], F32)
        kbias = consts.tile([P, NCI], F32)
        with tc.tile_pool(name="pbias", bufs=2, space="PSUM") as pb:
            for bias_dst, braw, wname in ((qbias, bq_col, "Wq"), (kbias, bk_col, "Wk")):
                for co in range(NCI):
                    psb = pb.tile([P, 1], F32, tag="psb")
                    for ci in range(NCI):
                        nc.tensor.matmul(
                            psb,
                            lhsT=w_bf[wname][:, ci, co * P : (co + 1) * P],
                            rhs=shift_bf[:, ci : ci + 1],
                            start=(ci == 0), stop=(ci == NCI - 1),
                        )
                    nc.vector.tensor_add(
                        out=bias_dst[:, co : co + 1], in0=braw[:, co : co + 1], in1=psb
                    )
            psr = pb.tile([1, C], F32, tag="psr")
            for ci in range(NCI):
                nc.tensor.matmul(
                    psr,
                    lhsT=shift_bf[:, ci : ci + 1],
                    rhs=w_bf["Wv"][:, ci, :],
                    start=(ci == 0), stop=(ci == NCI - 1),
                )
            bvn = stat.tile([1, C], F32, tag="bvn")
            nc.vector.tensor_add(out=bvn, in0=bv_row, in1=psr)
            nc.sync.dma_start(out=scl_sh_d[2, :], in_=bvn)
        bv_bc = consts.tile([P, C], F32)
        nc.gpsimd.dma_start(out=bv_bc, in_=scl_sh_d[2:3, :].to_broadcast((P, C)))

        # broadcast residual rows across partitions for the output phase
        scale_bc = consts.tile([P, C], F32)
        shift_bc = consts.tile([P, C], F32)
        nc.gpsimd.dma_start(out=scale_bc, in_=scl_sh_d[0:1, :].to_broadcast((P, C)))
        nc.gpsimd.dma_start(out=shift_bc, in_=scl_sh_d[1:2, :].to_broadcast((P, C)))

        # ---- phase 3: qT, kT [C, T] and v [T, C], all fp8 e4m3 ------------
        # (stored fp8 for the DoubleRow attention matmuls; bias added in f32
        # on the PSUM->SBUF copy)
        qT = consts.tile([P, NCI, T], FP8, tag="qT")
        kT = consts.tile([P, NCI, T], FP8, tag="kT")
        v_all = consts.tile([P, NJ, C], FP8, tag="v")
        with tc.tile_pool(name="qkv_ps", bufs=6, space="PSUM") as qkv_ps:
            for wt, dst, bias in ((w_bf["Wq"], qT, qbias), (w_bf["Wk"], kT, kbias)):
                for co in range(NCI):
                    for ichk in range(NI):
                        ps = qkv_ps.tile([P, 512], F32, tag="ps")
                        for ci in range(NCI):
                            nc.tensor.matmul(
                                ps,
                                lhsT=wt[:, ci, co * P : (co + 1) * P],
                                rhs=xT[:, ci, ichk * 512 : (ichk + 1) * 512],
                                start=(ci == 0), stop=(ci == NCI - 1),
                            )
                        nc.scalar.activation(
                            out=dst[:, co, ichk * 512 : (ichk + 1) * 512], in_=ps,
                            func=mybir.ActivationFunctionType.Identity,
                            bias=bias[:, co : co + 1], scale=1.0,
                        )
            for jb in range(NJ):
                ps = qkv_ps.tile([P, 512], F32, tag="ps")
                for ci in range(NCI):
                    nc.tensor.matmul(
                        ps,
                        lhsT=xT[:, ci, jb * P : (jb + 1) * P],
                        rhs=w_bf["Wv"][:, ci, :],
                        start=(ci == 0), stop=(ci == NCI - 1),
                    )
                nc.vector.scalar_tensor_tensor(
                    out=v_all[:, jb, :], in0=ps, scalar=1.0, in1=bv_bc,
                    op0=mybir.AluOpType.mult, op1=mybir.AluOpType.add,
                )
        qkv_ctx.close()  # free xT, Wq/Wk/Wv stages before the attention phase

        # ---- phase 4: attention + output, per i-chunk of 512 tokens -------
        # scores/attn@v/denominator all run fp8 DoubleRow: contraction over
        # channel pairs (scores) and j-block pairs (attn@v) at 2 MACs/cell.
        ps_s = ctx.enter_context(tc.tile_pool(name="ps_s", bufs=3, space="PSUM"))
        ps_pd = ctx.enter_context(tc.tile_pool(name="ps_pd", bufs=1, space="PSUM"))
        epool = ctx.enter_context(tc.tile_pool(name="epool", bufs=6))
        att = ctx.enter_context(tc.tile_pool(name="att", bufs=2))
        outp = ctx.enter_context(tc.tile_pool(name="outp", bufs=3))

        for ichk in range(NI):
            isl = slice(ichk * 512, (ichk + 1) * 512)
            ppA = ps_pd.tile([P, NCI // 2, 512], F32, tag="ppA")
            ppB = ps_pd.tile([P, NCI // 2, 512], F32, tag="ppB")
            pp = (ppA, ppB)
            pd = ps_pd.tile([P, 512], F32, tag="pd")
            for jo in range(NJ // 2):
                etp = epool.tile([P, 2, 512], FP8, tag="etp")
                for jj in range(2):
                    jb = 2 * jo + jj
                    ps = ps_s.tile([P, 512], F32, tag="ss")
                    for cp in range(NCI // 2):
                        nc.tensor.matmul(
                            ps,
                            lhsT=kT[:, 2 * cp : 2 * cp + 2, jb * P : (jb + 1) * P],
                            rhs=qT[:, 2 * cp : 2 * cp + 2, isl],
                            start=(cp == 0), stop=(cp == NCI // 2 - 1),
                            perf_mode=DR,
                        )
                    nc.scalar.activation(
                        out=etp[:, jj, :], in_=ps,
                        func=mybir.ActivationFunctionType.Exp, scale=SCALE,
                    )
                for cv in range(NCI):
                    nc.tensor.matmul(
                        pp[cv // 2][:, cv % 2, :],
                        lhsT=v_all[:, 2 * jo : 2 * jo + 2, cv * P : (cv + 1) * P],
                        rhs=etp,
                        start=(jo == 0), stop=(jo == NJ // 2 - 1),
                        perf_mode=DR,
                    )
                nc.tensor.matmul(
                    pd, lhsT=ones8, rhs=etp,
                    start=(jo == 0), stop=(jo == NJ // 2 - 1),
                    perf_mode=DR,
                )
            # denominator is already broadcast across partitions in PSUM;
            # fast approximate reciprocal (denom ~ 4e3, well-conditioned)
            rb = att.tile([P, 512], F32, tag="rb")
            nc.vector.reciprocal_approx_fast(out=rb, in_=pd)
            # normalized projT (bf16) for the final matmul
            pj = att.tile([P, NCI, 512], BF16, tag="pj")
            for cv in range(NCI):
                nc.vector.tensor_mul(out=pj[:, cv, :], in0=pp[cv // 2][:, cv % 2, :], in1=rb)
            # final proj + residual, per token block of 128
            for tb in range(4):
                itb = ichk * 4 + tb
                r = slice(itb * P, (itb + 1) * P)
                po = ps_pd.tile([P, C], F32, tag="pd", name=f"po_{ichk}_{tb}")
                for cv in range(NCI):
                    nc.tensor.matmul(
                        po,
                        lhsT=pj[:, cv, tb * P : (tb + 1) * P],
                        rhs=w_bf["Wp"][:, cv, :],
                        start=(cv == 0), stop=(cv == NCI - 1),
                    )
                xre = outp.tile([P, C], F32, tag="xre")
                nc.sync.dma_start(out=xre, in_=x[r, :])
                t = outp.tile([P, C], F32, tag="t")
                nc.vector.tensor_mul(out=t, in0=xre, in1=scale_bc)
                nc.vector.tensor_add(out=t, in0=t, in1=shift_bc)
                ot = outp.tile([P, C], F32, tag="ot")
                nc.vector.scalar_tensor_tensor(
                    out=ot, in0=po, scalar=1.0, in1=t,
                    op0=mybir.AluOpType.mult, op1=mybir.AluOpType.add,
                )
                nc.sync.dma_start(out=out[r, :], in_=ot)


def _install_neff_cache():
    """Memoize walrus NEFF compiles on a BIR content hash."""
    from concourse import bass2jax as b2j

    if getattr(b2j, "_neff_cache_installed", False):
        return
    cache_dir = os.environ.get("BASS_NEFF_CACHE", "/tmp/bass_neff_cache")
    os.makedirs(cache_dir, exist_ok=True)
    orig = b2j.compile_bir_kernel

    def cached(ant_bir_str, compile_dir_path, neff_name="file.neff"):
        key = hashlib.sha256(ant_bir_str).hexdigest()
        path = os.path.join(cache_dir, f"{key}.neff")
        if os.path.exists(path):
            dst = os.path.join(compile_dir_path, neff_name)
            with open(path, "rb") as f, open(dst, "wb") as g:
                g.write(f.read())
            return dst
        neff_file = orig(ant_bir_str, compile_dir_path, neff_name=neff_name)
        tmp = path + ".tmp"
        with open(neff_file, "rb") as f, open(tmp, "wb") as g:
            g.write(f.read())
        os.replace(tmp, path)
        return neff_file

    b2j.compile_bir_kernel = cached
    b2j._neff_cache_installed = True


_PROGRAM = None


def _get_program():
    global _PROGRAM
    if _PROGRAM is None:
        nc = bacc.Bacc("TRN2", debug=False)
        ins = {}
        for name in IN_NAMES:
            if name == "x":
                shape = [T, C]
            elif name.startswith("W"):
                shape = [C, C]
            else:
                shape = [C]
            ins[name] = nc.dram_tensor(name, shape, F32, kind="ExternalInput").ap()
        out = nc.dram_tensor("out", [T, C], F32, kind="ExternalOutput").ap()
        with tile.TileContext(nc) as tc:
            build_attn_tile(tc, {"out": out}, ins, T=T, C=C)
        nc.compile()
        _PROGRAM = nc
    return _PROGRAM


def _ensure_axon_hooks_stub():
    """bass_utils' trace path imports antenv.axon_hooks, which this image
    lacks; install a no-op registry so a stray BASS_TRACE env var degrades
    to 'no profile' instead of crashing."""
    import types

    try:
        import antenv.axon_hooks  # noqa: F401
        return
    except ImportError:
        pass
    import antenv

    mod = types.ModuleType("antenv.axon_hooks")
    _state = {"hook": None}
    mod.set_axon_ntff_profile_hook = lambda h: _state.__setitem__("hook", h)
    mod.get_axon_ntff_profile_hook = lambda: _state["hook"]
    sys.modules["antenv.axon_hooks"] = mod
    antenv.axon_hooks = mod


def run_spmd(inputs, trace=False):
    """Run the 8-core SPMD program; returns (out [B,H,W,C] f32, results obj)."""
    from concourse.bass_utils import run_bass_kernel_spmd

    _ensure_axon_hooks_stub()
    _install_neff_cache()
    nc = _get_program()
    arrs = {k: np.ascontiguousarray(np.asarray(v, dtype=np.float32)) for k, v in inputs.items()}
    x = arrs["x"].reshape(B, T, C)
    in_maps = []
    for b in range(B):
        m = {k: arrs[k] for k in IN_NAMES if k != "x"}
        m["x"] = x[b]
        in_maps.append(m)
    res = run_bass_kernel_spmd(nc, in_maps, core_ids=list(range(N_CORES)), trace=trace)
    out = np.stack([res.results[b]["out"] for b in range(B)], axis=0)
    return out.reshape(B, H, W, C).astype(np.float32), res


def kernel(**inputs) -> np.ndarray:
    out, _ = run_spmd(inputs, trace=False)
    return out


# revision 27
# speedup vs baseline: 1.0054x; 1.0054x over previous
"""nn_AttentionBlock TRN2 kernel: 8-way batch-parallel across 8 NeuronCores.

Each core runs an identical Bass/Tile program over one image of the batch
(B=8): instance-norm -> q/k/v projections -> full 4096x4096 attention ->
output projection + residual. No collectives are needed; inputs are
sharded over batch and the weights are replicated to every core.

Per-core program (T=4096 tokens, C=512 channels; qkv/proj matmuls bf16,
attention matmuls fp8-e4m3 DoubleRow, fp32 PSUM accumulation everywhere):
  - x tiles are transposed on the (otherwise idle) TensorEngine via
    identity matmuls into PSUM, then cast to xT [C, T] bf16 on ScalarE.
    bn_stats runs on the fp32 PSUM transposes to get per-channel stats.
  - The instance-norm affine (xn = x*scale + shift, per channel) is FOLDED
    into the q/k/v weights and biases: W' = diag(scale) @ W (per-partition
    row scaling), bias' = b + shift @ W. xT never needs normalizing.
  - qT/kT [C, T] via matmuls with W' stationary; v [T, C] natural.
  - scores computed transposed [j, i] = kT.T @ qT / sqrt(C); exp on ScalarE
    straight out of PSUM (no max subtraction -- |scores| < ~2 here).
  - attn @ v accumulated over j in PSUM [c, i]; a parallel ones-row matmul
    accumulates the softmax denominator; normalization is applied after the
    final Wp matmul by row scaling (linearity).
  - out = xn + proj + bp, with xn recomputed in fp32 from a re-read of x.
"""

import hashlib
import os
import sys

sys.path.insert(0, "/opt/trn_rl_repo")

from contextlib import ExitStack

import numpy as np

import concourse.bass as bass
import concourse.bacc as bacc
import concourse.tile as tile
from concourse import mybir
from concourse.masks import make_identity

F32 = mybir.dt.float32
BF16 = mybir.dt.bfloat16
FP8 = mybir.dt.float8e4
EPS = 1e-3
P = 128

B, H, W, C = 8, 64, 64, 512
T = H * W
N_CORES = 8

IN_NAMES = ("x", "gamma", "beta", "Wq", "bq", "Wk", "bk", "Wv", "bv", "Wp", "bp")


def build_attn_tile(tc, outs, ins, T=T, C=C):
    """Emit the per-core attention-block program into TileContext `tc`.

    outs: {"out": AP [T, C] f32}; ins: dict of APs keyed like setup_inputs().
    """
    nc = tc.nc
    x = ins["x"]
    out = outs["out"]
    NCI = C // P          # channel chunks of 128
    NT = T // P           # token blocks of 128
    NI = T // 512         # i-chunks of 512 tokens
    NJ = NT               # j-blocks of 128 tokens
    SCALE = float(C) ** -0.5
    DR = mybir.MatmulPerfMode.DoubleRow

    ctx = ExitStack()
    with ctx:
        consts = ctx.enter_context(tc.tile_pool(name="consts", bufs=1))
        dram = ctx.enter_context(tc.tile_pool(name="dram", bufs=1, space="DRAM"))

        # ---- tiny constants needed by phase 1 -----------------------------
        eps_col = consts.tile([P, 1], F32)
        nc.vector.memset(eps_col, EPS)
        ident = consts.tile([P, P], F32)
        make_identity(nc, ident)
        # all-ones fp8 stationary for the softmax-denominator matmul; with
        # M=128 every output partition gets the same column sum, so the
        # denominator lands pre-broadcast [128, 512] in PSUM.
        ones8 = consts.tile([P, 2, P], FP8)
        nc.vector.memset(ones8, 1.0)

        # phase-scoped pools: freed before the attention phase opens
        qkv_ctx = ExitStack()
        wqkv = qkv_ctx.enter_context(tc.tile_pool(name="wqkv", bufs=1))
        wstage = qkv_ctx.enter_context(tc.tile_pool(name="wstage", bufs=1))
        stat = qkv_ctx.enter_context(tc.tile_pool(name="stat", bufs=2))

        # ---- phase 1 FIRST (so its DMAs win the queue): load x tiles,
        # transpose on the TensorEngine, cast to xT bf16, bn_stats on the
        # fp32 PSUM transposes.
        xT = wqkv.tile([P, NCI, T], BF16, tag="xT")
        NG = NT // 4  # token-tile groups of 4 (one PSUM bank per channel chunk)
        st_all = stat.tile([P, NCI, NG, 6], F32, tag="st_all")
        with (
            tc.tile_pool(name="xin", bufs=3) as xin,
            tc.tile_pool(name="pst", bufs=2, space="PSUM") as pst,
        ):
            for g in range(NG):
                pts = [pst.tile([P, 512], F32, tag=f"pt{ci}", name=f"pt{ci}_{g}") for ci in range(NCI)]
                # one 1 MiB DMA per group of 4 token tiles (row = a*128 + p)
                xt = xin.tile([P, 4, C], F32, tag="xt")
                nc.sync.dma_start(
                    out=xt,
                    in_=x[g * 512 : (g + 1) * 512, :].rearrange("(a p) c -> p a c", p=P),
                )
                for s in range(4):
                    for ci in range(NCI):
                        nc.tensor.transpose(
                            pts[ci][:, s * P : (s + 1) * P],
                            xt[:, s, ci * P : (ci + 1) * P],
                            ident,
                        )
                for ci in range(NCI):
                    nc.scalar.copy(out=xT[:, ci, g * 512 : (g + 1) * 512], in_=pts[ci])
                    nc.vector.bn_stats(out=st_all[:, ci, g, :], in_=pts[ci])

        # ---- per-channel affine params / weights (off the critical path) --
        gam_col = consts.tile([P, NCI], F32)
        bet_col = consts.tile([P, NCI], F32)
        bq_col = consts.tile([P, NCI], F32)
        bk_col = consts.tile([P, NCI], F32)
        bp_col = consts.tile([P, NCI], F32)
        for ci in range(NCI):
            sl = slice(ci * P, (ci + 1) * P)
            nc.sync.dma_start(out=gam_col[:, ci : ci + 1], in_=ins["gamma"][sl])
            nc.sync.dma_start(out=bet_col[:, ci : ci + 1], in_=ins["beta"][sl])
            nc.sync.dma_start(out=bq_col[:, ci : ci + 1], in_=ins["bq"][sl])
            nc.sync.dma_start(out=bk_col[:, ci : ci + 1], in_=ins["bk"][sl])
            nc.sync.dma_start(out=bp_col[:, ci : ci + 1], in_=ins["bp"][sl])
        bv_row = consts.tile([1, C], F32)
        nc.sync.dma_start(out=bv_row, in_=ins["bv"].rearrange("(o c) -> o c", o=1))

        # stage Wq/Wk/Wv in f32 (scaled by the norm below); Wp casts now
        w_stage = {}
        for wname in ("Wq", "Wk", "Wv"):
            st = wstage.tile([P, NCI, C], F32, tag=f"stage_{wname}", name=f"stage_{wname}")
            w_stage[wname] = st
            for ci in range(NCI):
                nc.sync.dma_start(out=st[:, ci, :], in_=ins[wname][ci * P : (ci + 1) * P, :])
        w_bf = {}
        for wname in ("Wq", "Wk", "Wv"):
            w_bf[wname] = wqkv.tile([P, NCI, C], BF16, tag=f"w_{wname}", name=f"w_{wname}")
        w_bf["Wp"] = consts.tile([P, NCI, C], BF16, tag="w_Wp", name="w_Wp")
        with tc.tile_pool(name="wpstage", bufs=2) as wps:
            for ci in range(NCI):
                st = wps.tile([P, C], F32, tag="wps")
                nc.sync.dma_start(out=st, in_=ins["Wp"][ci * P : (ci + 1) * P, :])
                nc.vector.tensor_copy(out=w_bf["Wp"][:, ci, :], in_=st)

        # ---- stats -> per-channel scale/shift; fold into weights ---------
        # The whole istd/scale/shift chain runs vectorized over all NCI
        # channel chunks at once ([P, NCI] tiles) to shorten the serial DVE
        # chain that gates the first q matmul.
        scl_sh_d = dram.tile([3, C], F32)  # rows: scale, shift+bp, bv'
        scale_cols = consts.tile([P, NCI], F32)
        shift_bf = consts.tile([P, NCI], BF16)
        mv_all = stat.tile([P, NCI, 2], F32, tag="mv_all")
        for ci in range(NCI):
            nc.vector.bn_aggr(out=mv_all[:, ci, :], in_=st_all[:, ci])
        mean = mv_all[:, :, 0]
        var = mv_all[:, :, 1]
        # istd = 1/sqrt(var + eps), one Newton step on the inverse sqrt
        # (the ACT Sqrt spline has a loose precision budget).
        va = stat.tile([P, NCI], F32, tag="va")
        nc.vector.tensor_scalar_add(out=va, in0=var, scalar1=EPS)
        s0 = stat.tile([P, NCI], F32, tag="s0")
        nc.scalar.activation(
            out=s0, in_=var, func=mybir.ActivationFunctionType.Sqrt,
            bias=eps_col, scale=1.0,
        )
        r0 = stat.tile([P, NCI], F32, tag="r0")
        nc.vector.reciprocal(out=r0, in_=s0)
        t0 = stat.tile([P, NCI], F32, tag="t0")
        nc.vector.tensor_mul(out=t0, in0=r0, in1=r0)
        nc.vector.tensor_mul(out=t0, in0=t0, in1=va)
        nc.vector.tensor_scalar(
            out=t0, in0=t0, scalar1=-0.5, scalar2=1.5,
            op0=mybir.AluOpType.mult, op1=mybir.AluOpType.add,
        )
        istd = stat.tile([P, NCI], F32, tag="istd")
        nc.vector.tensor_mul(out=istd, in0=r0, in1=t0)

        nc.vector.tensor_mul(out=scale_cols, in0=gam_col, in1=istd)
        shift_c = stat.tile([P, NCI], F32, tag="shift_c")
        nc.vector.tensor_mul(out=shift_c, in0=mean, in1=scale_cols)
        nc.vector.tensor_sub(out=shift_c, in0=bet_col, in1=shift_c)
        nc.vector.tensor_copy(out=shift_bf, in_=shift_c)
        sh2 = stat.tile([P, NCI], F32, tag="sh2")
        nc.vector.tensor_add(out=sh2, in0=shift_c, in1=bp_col)
        nc.sync.dma_start(
            out=scl_sh_d[0].rearrange("(a p) -> p a", p=P), in_=scale_cols
        )
        nc.sync.dma_start(out=scl_sh_d[1].rearrange("(a p) -> p a", p=P), in_=sh2)

        # W' = diag(scale) @ W for q/k/v (per-partition row scale), bf16
        for wname in ("Wq", "Wk", "Wv"):
            st = w_stage[wname]
            for ci in range(NCI):
                nc.vector.tensor_scalar_mul(
                    out=w_bf[wname][:, ci, :], in0=st[:, ci, :],
                    scalar1=scale_cols[:, ci : ci + 1],
                )

        # folded biases: bias' = b + shift @ W'
        qbias = consts.tile([P, NCI], F32)
        kbias = consts.tile([P, NCI], F32)
        with tc.tile_pool(name="pbias", bufs=2, space="PSUM") as pb:
            for bias_dst, braw, wname in ((qbias, bq_col, "Wq"), (kbias, bk_col, "Wk")):
                for co in range(NCI):
                    psb = pb.tile([P, 1], F32, tag="psb")
                    for ci in range(NCI):
                        nc.tensor.matmul(
                            psb,
                            lhsT=w_bf[wname][:, ci, co * P : (co + 1) * P],
                            rhs=shift_bf[:, ci : ci + 1],
                            start=(ci == 0), stop=(ci == NCI - 1),
                        )
                    nc.vector.tensor_add(
                        out=bias_dst[:, co : co + 1], in0=braw[:, co : co + 1], in1=psb
                    )
            psr = pb.tile([1, C], F32, tag="psr")
            for ci in range(NCI):
                nc.tensor.matmul(
                    psr,
                    lhsT=shift_bf[:, ci : ci + 1],
                    rhs=w_bf["Wv"][:, ci, :],
                    start=(ci == 0), stop=(ci == NCI - 1),
                )
            bvn = stat.tile([1, C], F32, tag="bvn")
            nc.vector.tensor_add(out=bvn, in0=bv_row, in1=psr)
            nc.sync.dma_start(out=scl_sh_d[2, :], in_=bvn)
        bv_bc = consts.tile([P, C], F32)
        nc.gpsimd.dma_start(out=bv_bc, in_=scl_sh_d[2:3, :].to_broadcast((P, C)))

        # broadcast residual rows across partitions for the output phase
        scale_bc = consts.tile([P, C], F32)
        shift_bc = consts.tile([P, C], F32)
        nc.gpsimd.dma_start(out=scale_bc, in_=scl_sh_d[0:1, :].to_broadcast((P, C)))
        nc.gpsimd.dma_start(out=shift_bc, in_=scl_sh_d[1:2, :].to_broadcast((P, C)))

        # ---- phase 3: qT, kT [C, T] and v [T, C], all fp8 e4m3 ------------
        # (stored fp8 for the DoubleRow attention matmuls; bias added in f32
        # on the PSUM->SBUF copy)
        qT = consts.tile([P, NCI, T], FP8, tag="qT")
        kT = consts.tile([P, NCI, T], FP8, tag="kT")
        v_all = consts.tile([P, NJ, C], FP8, tag="v")
        with tc.tile_pool(name="qkv_ps", bufs=6, space="PSUM") as qkv_ps:
            for wt, dst, bias in ((w_bf["Wq"], qT, qbias), (w_bf["Wk"], kT, kbias)):
                for co in range(NCI):
                    for ichk in range(NI):
                        ps = qkv_ps.tile([P, 512], F32, tag="ps")
                        for ci in range(NCI):
                            nc.tensor.matmul(
                                ps,
                                lhsT=wt[:, ci, co * P : (co + 1) * P],
                                rhs=xT[:, ci, ichk * 512 : (ichk + 1) * 512],
                                start=(ci == 0), stop=(ci == NCI - 1),
                            )
                        nc.scalar.activation(
                            out=dst[:, co, ichk * 512 : (ichk + 1) * 512], in_=ps,
                            func=mybir.ActivationFunctionType.Identity,
                            bias=bias[:, co : co + 1], scale=1.0,
                        )
            for jb in range(NJ):
                ps = qkv_ps.tile([P, 512], F32, tag="ps")
                for ci in range(NCI):
                    nc.tensor.matmul(
                        ps,
                        lhsT=xT[:, ci, jb * P : (jb + 1) * P],
                        rhs=w_bf["Wv"][:, ci, :],
                        start=(ci == 0), stop=(ci == NCI - 1),
                    )
                nc.vector.scalar_tensor_tensor(
                    out=v_all[:, jb, :], in0=ps, scalar=1.0, in1=bv_bc,
                    op0=mybir.AluOpType.mult, op1=mybir.AluOpType.add,
                )
        qkv_ctx.close()  # free xT, Wq/Wk/Wv stages before the attention phase

        # ---- phase 4: attention + output, per i-chunk of 512 tokens -------
        # scores/attn@v/denominator all run fp8 DoubleRow: contraction over
        # channel pairs (scores) and j-block pairs (attn@v) at 2 MACs/cell.
        ps_s = ctx.enter_context(tc.tile_pool(name="ps_s", bufs=3, space="PSUM"))
        ps_pd = ctx.enter_context(tc.tile_pool(name="ps_pd", bufs=1, space="PSUM"))
        epool = ctx.enter_context(tc.tile_pool(name="epool", bufs=6))
        att = ctx.enter_context(tc.tile_pool(name="att", bufs=3))
        outp = ctx.enter_context(tc.tile_pool(name="outp", bufs=3))

        for ichk in range(NI):
            isl = slice(ichk * 512, (ichk + 1) * 512)
            ppA = ps_pd.tile([P, NCI // 2, 512], F32, tag="ppA")
            ppB = ps_pd.tile([P, NCI // 2, 512], F32, tag="ppB")
            pp = (ppA, ppB)
            pd = ps_pd.tile([P, 512], F32, tag="pd")
            for jo in range(NJ // 2):
                etp = epool.tile([P, 2, 512], FP8, tag="etp")
                for jj in range(2):
                    jb = 2 * jo + jj
                    ps = ps_s.tile([P, 512], F32, tag="ss")
                    for cp in range(NCI // 2):
                        nc.tensor.matmul(
                            ps,
                            lhsT=kT[:, 2 * cp : 2 * cp + 2, jb * P : (jb + 1) * P],
                            rhs=qT[:, 2 * cp : 2 * cp + 2, isl],
                            start=(cp == 0), stop=(cp == NCI // 2 - 1),
                            perf_mode=DR,
                        )
                    nc.scalar.activation(
                        out=etp[:, jj, :], in_=ps,
                        func=mybir.ActivationFunctionType.Exp, scale=SCALE,
                    )
                for cv in range(NCI):
                    nc.tensor.matmul(
                        pp[cv // 2][:, cv % 2, :],
                        lhsT=v_all[:, 2 * jo : 2 * jo + 2, cv * P : (cv + 1) * P],
                        rhs=etp,
                        start=(jo == 0), stop=(jo == NJ // 2 - 1),
                        perf_mode=DR,
                    )
                nc.tensor.matmul(
                    pd, lhsT=ones8, rhs=etp,
                    start=(jo == 0), stop=(jo == NJ // 2 - 1),
                    perf_mode=DR,
                )
            # denominator is already broadcast across partitions in PSUM;
            # fast approximate reciprocal (denom ~ 4e3, well-conditioned)
            rb = att.tile([P, 512], F32, tag="rb")
            nc.vector.reciprocal_approx_fast(out=rb, in_=pd)
            # normalized projT (bf16) for the final matmul
            pj = att.tile([P, NCI, 512], BF16, tag="pj")
            for cv in range(NCI):
                nc.vector.tensor_mul(out=pj[:, cv, :], in0=pp[cv // 2][:, cv % 2, :], in1=rb)
            # final proj + residual, per token block of 128
            for tb in range(4):
                itb = ichk * 4 + tb
                r = slice(itb * P, (itb + 1) * P)
                po = ps_pd.tile([P, C], F32, tag="pd", name=f"po_{ichk}_{tb}")
                for cv in range(NCI):
                    nc.tensor.matmul(
                        po,
                        lhsT=pj[:, cv, tb * P : (tb + 1) * P],
                        rhs=w_bf["Wp"][:, cv, :],
                        start=(cv == 0), stop=(cv == NCI - 1),
                    )
                xre = outp.tile([P, C], F32, tag="xre")
                nc.sync.dma_start(out=xre, in_=x[r, :])
                t = outp.tile([P, C], F32, tag="t")
                nc.vector.tensor_mul(out=t, in0=xre, in1=scale_bc)
                nc.vector.tensor_add(out=t, in0=t, in1=shift_bc)
                ot = outp.tile([P, C], F32, tag="ot")
                nc.vector.scalar_tensor_tensor(
                    out=ot, in0=po, scalar=1.0, in1=t,
                    op0=mybir.AluOpType.mult, op1=mybir.AluOpType.add,
                )
                nc.sync.dma_start(out=out[r, :], in_=ot)


def _install_neff_cache():
    """Memoize walrus NEFF compiles on a BIR content hash."""
    from concourse import bass2jax as b2j

    if getattr(b2j, "_neff_cache_installed", False):
        return
    cache_dir = os.environ.get("BASS_NEFF_CACHE", "/tmp/bass_neff_cache")
    os.makedirs(cache_dir, exist_ok=True)
    orig = b2j.compile_bir_kernel

    def cached(ant_bir_str, compile_dir_path, neff_name="file.neff"):
        key = hashlib.sha256(ant_bir_str).hexdigest()
        path = os.path.join(cache_dir, f"{key}.neff")
        if os.path.exists(path):
            dst = os.path.join(compile_dir_path, neff_name)
            with open(path, "rb") as f, open(dst, "wb") as g:
                g.write(f.read())
            return dst
        neff_file = orig(ant_bir_str, compile_dir_path, neff_name=neff_name)
        tmp = path + ".tmp"
        with open(neff_file, "rb") as f, open(tmp, "wb") as g:
            g.write(f.read())
        os.replace(tmp, path)
        return neff_file

    b2j.compile_bir_kernel = cached
    b2j._neff_cache_installed = True


_PROGRAM = None


def _get_program():
    global _PROGRAM
    if _PROGRAM is None:
        nc = bacc.Bacc("TRN2", debug=False)
        ins = {}
        for name in IN_NAMES:
            if name == "x":
                shape = [T, C]
            elif name.startswith("W"):
                shape = [C, C]
            else:
                shape = [C]
            ins[name] = nc.dram_tensor(name, shape, F32, kind="ExternalInput").ap()
        out = nc.dram_tensor("out", [T, C], F32, kind="ExternalOutput").ap()
        with tile.TileContext(nc) as tc:
            build_attn_tile(tc, {"out": out}, ins, T=T, C=C)
        nc.compile()
        _PROGRAM = nc
    return _PROGRAM


def _ensure_axon_hooks_stub():
    """bass_utils' trace path imports antenv.axon_hooks, which this image
    lacks; install a no-op registry so a stray BASS_TRACE env var degrades
    to 'no profile' instead of crashing."""
    import types

    try:
        import antenv.axon_hooks  # noqa: F401
        return
    except ImportError:
        pass
    import antenv

    mod = types.ModuleType("antenv.axon_hooks")
    _state = {"hook": None}
    mod.set_axon_ntff_profile_hook = lambda h: _state.__setitem__("hook", h)
    mod.get_axon_ntff_profile_hook = lambda: _state["hook"]
    sys.modules["antenv.axon_hooks"] = mod
    antenv.axon_hooks = mod


def run_spmd(inputs, trace=False):
    """Run the 8-core SPMD program; returns (out [B,H,W,C] f32, results obj)."""
    from concourse.bass_utils import run_bass_kernel_spmd

    _ensure_axon_hooks_stub()
    _install_neff_cache()
    nc = _get_program()
    arrs = {k: np.ascontiguousarray(np.asarray(v, dtype=np.float32)) for k, v in inputs.items()}
    x = arrs["x"].reshape(B, T, C)
    in_maps = []
    for b in range(B):
        m = {k: arrs[k] for k in IN_NAMES if k != "x"}
        m["x"] = x[b]
        in_maps.append(m)
    res = run_bass_kernel_spmd(nc, in_maps, core_ids=list(range(N_CORES)), trace=trace)
    out = np.stack([res.results[b]["out"] for b in range(B)], axis=0)
    return out.reshape(B, H, W, C).astype(np.float32), res


def kernel(**inputs) -> np.ndarray:
    out, _ = run_spmd(inputs, trace=False)
    return out


# revision 28
# speedup vs baseline: 1.1964x; 1.1900x over previous
"""nn_AttentionBlock TRN2 kernel: 8-way batch-parallel across 8 NeuronCores.

Each core runs an identical Bass/Tile program over one image of the batch
(B=8): instance-norm -> q/k/v projections -> full 4096x4096 attention ->
output projection + residual. No collectives are needed; inputs are
sharded over batch and the weights are replicated to every core.

Per-core program (T=4096 tokens, C=512 channels; qkv/proj matmuls bf16,
attention matmuls fp8-e4m3 DoubleRow, fp32 PSUM accumulation everywhere):
  - x tiles are transposed on the (otherwise idle) TensorEngine via
    identity matmuls into PSUM, then cast to xT [C, T] bf16 on ScalarE.
    bn_stats runs on the fp32 PSUM transposes to get per-channel stats.
  - The instance-norm affine (xn = x*scale + shift, per channel) is FOLDED
    into the q/k/v weights and biases: W' = diag(scale) @ W (per-partition
    row scaling), bias' = b + shift @ W. xT never needs normalizing.
  - qT/kT [C, T] via matmuls with W' stationary; v [T, C] natural.
  - scores computed transposed [j, i] = kT.T @ qT / sqrt(C); exp on ScalarE
    straight out of PSUM (no max subtraction -- |scores| < ~2 here).
  - attn @ v accumulated over j in PSUM [c, i]; a parallel ones-row matmul
    accumulates the softmax denominator; normalization is applied after the
    final Wp matmul by row scaling (linearity).
  - out = xn + proj + bp, with xn recomputed in fp32 from a re-read of x.
"""

import hashlib
import os
import sys

sys.path.insert(0, "/opt/trn_rl_repo")

from contextlib import ExitStack

import numpy as np

import concourse.bass as bass
import concourse.bacc as bacc
import concourse.tile as tile
from concourse import mybir
from concourse.masks import make_identity

F32 = mybir.dt.float32
BF16 = mybir.dt.bfloat16
FP8 = mybir.dt.float8e4
EPS = 1e-3
P = 128

B, H, W, C = 8, 64, 64, 512
T = H * W
N_CORES = 8

IN_NAMES = ("x", "gamma", "beta", "Wq", "bq", "Wk", "bk", "Wv", "bv", "Wp", "bp")


def build_attn_tile(tc, outs, ins, T=T, C=C):
    """Emit the per-core attention-block program into TileContext `tc`.

    outs: {"out": AP [T, C] f32}; ins: dict of APs keyed like setup_inputs().
    """
    nc = tc.nc
    x = ins["x"]
    out = outs["out"]
    NCI = C // P          # channel chunks of 128
    NT = T // P           # token blocks of 128
    NI = T // 512         # i-chunks of 512 tokens
    NJ = NT               # j-blocks of 128 tokens
    SCALE = float(C) ** -0.5
    DR = mybir.MatmulPerfMode.DoubleRow

    ctx = ExitStack()
    with ctx:
        consts = ctx.enter_context(tc.tile_pool(name="consts", bufs=1))
        dram = ctx.enter_context(tc.tile_pool(name="dram", bufs=1, space="DRAM"))

        # ---- tiny constants needed by phase 1 -----------------------------
        eps_col = consts.tile([P, 1], F32)
        nc.vector.memset(eps_col, EPS)
        ident = consts.tile([P, P], F32)
        make_identity(nc, ident)
        # all-ones fp8 stationary for the softmax-denominator matmul; with
        # M=128 every output partition gets the same column sum, so the
        # denominator lands pre-broadcast [128, 512] in PSUM.
        ones8 = consts.tile([P, 2, P], FP8)
        nc.vector.memset(ones8, 1.0)

        # phase-scoped pools: freed before the attention phase opens
        qkv_ctx = ExitStack()
        wqkv = qkv_ctx.enter_context(tc.tile_pool(name="wqkv", bufs=1))
        wstage = qkv_ctx.enter_context(tc.tile_pool(name="wstage", bufs=1))
        stat = qkv_ctx.enter_context(tc.tile_pool(name="stat", bufs=2))

        # ---- phase 1 FIRST (so its DMAs win the queue): load x tiles,
        # transpose on the TensorEngine, cast to xT bf16, bn_stats on the
        # fp32 PSUM transposes.
        xT = wqkv.tile([P, NCI, T], BF16, tag="xT")
        NG = NT // 4  # token-tile groups of 4 (one PSUM bank per channel chunk)
        st_all = stat.tile([P, NCI, NG, 6], F32, tag="st_all")
        with (
            tc.tile_pool(name="xin", bufs=3) as xin,
            tc.tile_pool(name="pst", bufs=2, space="PSUM") as pst,
        ):
            for g in range(NG):
                pts = [pst.tile([P, 512], F32, tag=f"pt{ci}", name=f"pt{ci}_{g}") for ci in range(NCI)]
                # one 1 MiB DMA per group of 4 token tiles (row = a*128 + p)
                xt = xin.tile([P, 4, C], F32, tag="xt")
                nc.sync.dma_start(
                    out=xt,
                    in_=x[g * 512 : (g + 1) * 512, :].rearrange("(a p) c -> p a c", p=P),
                )
                for s in range(4):
                    for ci in range(NCI):
                        nc.tensor.transpose(
                            pts[ci][:, s * P : (s + 1) * P],
                            xt[:, s, ci * P : (ci + 1) * P],
                            ident,
                        )
                for ci in range(NCI):
                    nc.scalar.copy(out=xT[:, ci, g * 512 : (g + 1) * 512], in_=pts[ci])
                    nc.vector.bn_stats(out=st_all[:, ci, g, :], in_=pts[ci])

        # ---- per-channel affine params / weights (off the critical path) --
        gam_col = consts.tile([P, NCI], F32)
        bet_col = consts.tile([P, NCI], F32)
        bq_col = consts.tile([P, NCI], F32)
        bk_col = consts.tile([P, NCI], F32)
        bp_col = consts.tile([P, NCI], F32)
        for ci in range(NCI):
            sl = slice(ci * P, (ci + 1) * P)
            nc.sync.dma_start(out=gam_col[:, ci : ci + 1], in_=ins["gamma"][sl])
            nc.sync.dma_start(out=bet_col[:, ci : ci + 1], in_=ins["beta"][sl])
            nc.sync.dma_start(out=bq_col[:, ci : ci + 1], in_=ins["bq"][sl])
            nc.sync.dma_start(out=bk_col[:, ci : ci + 1], in_=ins["bk"][sl])
            nc.sync.dma_start(out=bp_col[:, ci : ci + 1], in_=ins["bp"][sl])
        bv_row = consts.tile([1, C], F32)
        nc.sync.dma_start(out=bv_row, in_=ins["bv"].rearrange("(o c) -> o c", o=1))

        # stage Wq/Wk/Wv in f32 (scaled by the norm below); Wp casts now
        w_stage = {}
        for wname in ("Wq", "Wk", "Wv"):
            st = wstage.tile([P, NCI, C], F32, tag=f"stage_{wname}", name=f"stage_{wname}")
            w_stage[wname] = st
            for ci in range(NCI):
                nc.sync.dma_start(out=st[:, ci, :], in_=ins[wname][ci * P : (ci + 1) * P, :])
        w_bf = {}
        for wname in ("Wq", "Wk", "Wv"):
            w_bf[wname] = wqkv.tile([P, NCI, C], BF16, tag=f"w_{wname}", name=f"w_{wname}")
        w_bf["Wp"] = consts.tile([P, NCI, C], BF16, tag="w_Wp", name="w_Wp")
        with tc.tile_pool(name="wpstage", bufs=2) as wps:
            for ci in range(NCI):
                st = wps.tile([P, C], F32, tag="wps")
                nc.sync.dma_start(out=st, in_=ins["Wp"][ci * P : (ci + 1) * P, :])
                nc.vector.tensor_copy(out=w_bf["Wp"][:, ci, :], in_=st)

        # ---- stats -> per-channel scale/shift; fold into weights ---------
        # The whole istd/scale/shift chain runs vectorized over all NCI
        # channel chunks at once ([P, NCI] tiles) to shorten the serial DVE
        # chain that gates the first q matmul.
        scl_sh_d = dram.tile([3, C], F32)  # rows: scale, shift+bp, bv'
        scale_cols = consts.tile([P, NCI], F32)
        shift_bf = consts.tile([P, NCI], BF16)
        mv_all = stat.tile([P, NCI, 2], F32, tag="mv_all")
        for ci in range(NCI):
            nc.vector.bn_aggr(out=mv_all[:, ci, :], in_=st_all[:, ci])
        mean = mv_all[:, :, 0]
        var = mv_all[:, :, 1]
        # istd = 1/sqrt(var + eps), one Newton step on the inverse sqrt
        # (the ACT Sqrt spline has a loose precision budget).
        va = stat.tile([P, NCI], F32, tag="va")
        nc.vector.tensor_scalar_add(out=va, in0=var, scalar1=EPS)
        s0 = stat.tile([P, NCI], F32, tag="s0")
        nc.scalar.activation(
            out=s0, in_=var, func=mybir.ActivationFunctionType.Sqrt,
            bias=eps_col, scale=1.0,
        )
        r0 = stat.tile([P, NCI], F32, tag="r0")
        nc.vector.reciprocal(out=r0, in_=s0)
        t0 = stat.tile([P, NCI], F32, tag="t0")
        nc.vector.tensor_mul(out=t0, in0=r0, in1=r0)
        nc.vector.tensor_mul(out=t0, in0=t0, in1=va)
        nc.vector.tensor_scalar(
            out=t0, in0=t0, scalar1=-0.5, scalar2=1.5,
            op0=mybir.AluOpType.mult, op1=mybir.AluOpType.add,
        )
        istd = stat.tile([P, NCI], F32, tag="istd")
        nc.vector.tensor_mul(out=istd, in0=r0, in1=t0)

        nc.vector.tensor_mul(out=scale_cols, in0=gam_col, in1=istd)
        shift_c = stat.tile([P, NCI], F32, tag="shift_c")
        nc.vector.tensor_mul(out=shift_c, in0=mean, in1=scale_cols)
        nc.vector.tensor_sub(out=shift_c, in0=bet_col, in1=shift_c)
        nc.vector.tensor_copy(out=shift_bf, in_=shift_c)
        sh2 = stat.tile([P, NCI], F32, tag="sh2")
        nc.vector.tensor_add(out=sh2, in0=shift_c, in1=bp_col)
        nc.sync.dma_start(
            out=scl_sh_d[0].rearrange("(a p) -> p a", p=P), in_=scale_cols
        )
        nc.sync.dma_start(out=scl_sh_d[1].rearrange("(a p) -> p a", p=P), in_=sh2)

        # W' = diag(scale) @ W for q/k/v (per-partition row scale), bf16
        for wname in ("Wq", "Wk", "Wv"):
            st = w_stage[wname]
            for ci in range(NCI):
                nc.vector.tensor_scalar_mul(
                    out=w_bf[wname][:, ci, :], in0=st[:, ci, :],
                    scalar1=scale_cols[:, ci : ci + 1],
                )

        # folded biases: bias' = b + shift @ W'
        qbias = consts.tile([P, NCI], F32)
        kbias = consts.tile([P, NCI], F32)
        with tc.tile_pool(name="pbias", bufs=2, space="PSUM") as pb:
            for bias_dst, braw, wname in ((qbias, bq_col, "Wq"), (kbias, bk_col, "Wk")):
                for co in range(NCI):
                    psb = pb.tile([P, 1], F32, tag="psb")
                    for ci in range(NCI):
                        nc.tensor.matmul(
                            psb,
                            lhsT=w_bf[wname][:, ci, co * P : (co + 1) * P],
                            rhs=shift_bf[:, ci : ci + 1],
                            start=(ci == 0), stop=(ci == NCI - 1),
                        )
                    nc.vector.tensor_add(
                        out=bias_dst[:, co : co + 1], in0=braw[:, co : co + 1], in1=psb
                    )
            psr = pb.tile([1, C], F32, tag="psr")
            for ci in range(NCI):
                nc.tensor.matmul(
                    psr,
                    lhsT=shift_bf[:, ci : ci + 1],
                    rhs=w_bf["Wv"][:, ci, :],
                    start=(ci == 0), stop=(ci == NCI - 1),
                )
            bvn = stat.tile([1, C], F32, tag="bvn")
            nc.vector.tensor_add(out=bvn, in0=bv_row, in1=psr)
            nc.sync.dma_start(out=scl_sh_d[2, :], in_=bvn)
        bv_bc = consts.tile([P, C], F32)
        nc.gpsimd.dma_start(out=bv_bc, in_=scl_sh_d[2:3, :].to_broadcast((P, C)))

        # broadcast residual rows across partitions for the output phase
        scale_bc = consts.tile([P, C], F32)
        shift_bc = consts.tile([P, C], F32)
        nc.gpsimd.dma_start(out=scale_bc, in_=scl_sh_d[0:1, :].to_broadcast((P, C)))
        nc.gpsimd.dma_start(out=shift_bc, in_=scl_sh_d[1:2, :].to_broadcast((P, C)))

        # ---- phase 3: qT, kT [C, T] and v [T, C], all fp8 e4m3 ------------
        # (stored fp8 for the DoubleRow attention matmuls; bias added in f32
        # on the PSUM->SBUF copy)
        qT = consts.tile([P, NCI, T], FP8, tag="qT")
        kT = consts.tile([P, NCI, T], FP8, tag="kT")
        v_all = consts.tile([P, NJ, C], FP8, tag="v")
        with tc.tile_pool(name="qkv_ps", bufs=6, space="PSUM") as qkv_ps:
            for wt, dst, bias in ((w_bf["Wq"], qT, qbias), (w_bf["Wk"], kT, kbias)):
                for co in range(NCI):
                    for ichk in range(NI):
                        ps = qkv_ps.tile([P, 512], F32, tag="ps")
                        for ci in range(NCI):
                            nc.tensor.matmul(
                                ps,
                                lhsT=wt[:, ci, co * P : (co + 1) * P],
                                rhs=xT[:, ci, ichk * 512 : (ichk + 1) * 512],
                                start=(ci == 0), stop=(ci == NCI - 1),
                            )
                        nc.scalar.activation(
                            out=dst[:, co, ichk * 512 : (ichk + 1) * 512], in_=ps,
                            func=mybir.ActivationFunctionType.Identity,
                            bias=bias[:, co : co + 1], scale=1.0,
                        )
            for jb in range(NJ):
                ps = qkv_ps.tile([P, 512], F32, tag="ps")
                for ci in range(NCI):
                    nc.tensor.matmul(
                        ps,
                        lhsT=xT[:, ci, jb * P : (jb + 1) * P],
                        rhs=w_bf["Wv"][:, ci, :],
                        start=(ci == 0), stop=(ci == NCI - 1),
                    )
                nc.vector.scalar_tensor_tensor(
                    out=v_all[:, jb, :], in0=ps, scalar=1.0, in1=bv_bc,
                    op0=mybir.AluOpType.mult, op1=mybir.AluOpType.add,
                )
        qkv_ctx.close()  # free xT, Wq/Wk/Wv stages before the attention phase

        # ---- phase 4: attention + output, per i-chunk of 512 tokens -------
        # scores/attn@v/denominator all run fp8 DoubleRow: contraction over
        # channel pairs (scores) and j-block pairs (attn@v) at 2 MACs/cell.
        ps_s = ctx.enter_context(tc.tile_pool(name="ps_s", bufs=3, space="PSUM"))
        ps_pd = ctx.enter_context(tc.tile_pool(name="ps_pd", bufs=1, space="PSUM"))
        epool = ctx.enter_context(tc.tile_pool(name="epool", bufs=8))
        att = ctx.enter_context(tc.tile_pool(name="att", bufs=3))
        outp = ctx.enter_context(tc.tile_pool(name="outp", bufs=3))

        for ichk in range(NI):
            isl = slice(ichk * 512, (ichk + 1) * 512)
            ppA = ps_pd.tile([P, NCI // 2, 512], F32, tag="ppA")
            ppB = ps_pd.tile([P, NCI // 2, 512], F32, tag="ppB")
            pp = (ppA, ppB)
            pd = ps_pd.tile([P, 512], F32, tag="pd")
            for jo in range(NJ // 2):
                etp = epool.tile([P, 2, 512], FP8, tag="etp")
                for jj in range(2):
                    jb = 2 * jo + jj
                    ps = ps_s.tile([P, 512], F32, tag="ss")
                    for cp in range(NCI // 2):
                        nc.tensor.matmul(
                            ps,
                            lhsT=kT[:, 2 * cp : 2 * cp + 2, jb * P : (jb + 1) * P],
                            rhs=qT[:, 2 * cp : 2 * cp + 2, isl],
                            start=(cp == 0), stop=(cp == NCI // 2 - 1),
                            perf_mode=DR,
                        )
                    nc.scalar.activation(
                        out=etp[:, jj, :], in_=ps,
                        func=mybir.ActivationFunctionType.Exp, scale=SCALE,
                    )
                # denominator first: at the last j-pair its PSUM completes
                # before the attn@v matmuls, so the reciprocal overlaps them
                nc.tensor.matmul(
                    pd, lhsT=ones8, rhs=etp,
                    start=(jo == 0), stop=(jo == NJ // 2 - 1),
                    perf_mode=DR,
                )
                for cv in range(NCI):
                    nc.tensor.matmul(
                        pp[cv // 2][:, cv % 2, :],
                        lhsT=v_all[:, 2 * jo : 2 * jo + 2, cv * P : (cv + 1) * P],
                        rhs=etp,
                        start=(jo == 0), stop=(jo == NJ // 2 - 1),
                        perf_mode=DR,
                    )
            # denominator is already broadcast across partitions in PSUM;
            # fast approximate reciprocal (denom ~ 4e3, well-conditioned)
            rb = att.tile([P, 512], F32, tag="rb")
            nc.vector.reciprocal_approx_fast(out=rb, in_=pd)
            # normalized projT (bf16) for the final matmul
            pj = att.tile([P, NCI, 512], BF16, tag="pj")
            for cv in range(NCI):
                nc.vector.tensor_mul(out=pj[:, cv, :], in0=pp[cv // 2][:, cv % 2, :], in1=rb)
            # final proj + residual, per token block of 128
            for tb in range(4):
                itb = ichk * 4 + tb
                r = slice(itb * P, (itb + 1) * P)
                po = ps_pd.tile([P, C], F32, tag="pd", name=f"po_{ichk}_{tb}")
                for cv in range(NCI):
                    nc.tensor.matmul(
                        po,
                        lhsT=pj[:, cv, tb * P : (tb + 1) * P],
                        rhs=w_bf["Wp"][:, cv, :],
                        start=(cv == 0), stop=(cv == NCI - 1),
                    )
                xre = outp.tile([P, C], F32, tag="xre")
                nc.sync.dma_start(out=xre, in_=x[r, :])
                t = outp.tile([P, C], F32, tag="t")
                nc.vector.tensor_mul(out=t, in0=xre, in1=scale_bc)
                nc.vector.tensor_add(out=t, in0=t, in1=shift_bc)
                ot = outp.tile([P, C], F32, tag="ot")
                nc.vector.scalar_tensor_tensor(
                    out=ot, in0=po, scalar=1.0, in1=t,
                    op0=mybir.AluOpType.mult, op1=mybir.AluOpType.add,
                )
                nc.sync.dma_start(out=out[r, :], in_=ot)


def _install_neff_cache():
    """Memoize walrus NEFF compiles on a BIR content hash."""
    from concourse import bass2jax as b2j

    if getattr(b2j, "_neff_cache_installed", False):
        return
    cache_dir = os.environ.get("BASS_NEFF_CACHE", "/tmp/bass_neff_cache")
    os.makedirs(cache_dir, exist_ok=True)
    orig = b2j.compile_bir_kernel

    def cached(ant_bir_str, compile_dir_path, neff_name="file.neff"):
        key = hashlib.sha256(ant_bir_str).hexdigest()
        path = os.path.join(cache_dir, f"{key}.neff")
        if os.path.exists(path):
            dst = os.path.join(compile_dir_path, neff_name)
            with open(path, "rb") as f, open(dst, "wb") as g:
                g.write(f.read())
            return dst
        neff_file = orig(ant_bir_str, compile_dir_path, neff_name=neff_name)
        tmp = path + ".tmp"
        with open(neff_file, "rb") as f, open(tmp, "wb") as g:
            g.write(f.read())
        os.replace(tmp, path)
        return neff_file

    b2j.compile_bir_kernel = cached
    b2j._neff_cache_installed = True


_PROGRAM = None


def _get_program():
    global _PROGRAM
    if _PROGRAM is None:
        nc = bacc.Bacc("TRN2", debug=False)
        ins = {}
        for name in IN_NAMES:
            if name == "x":
                shape = [T, C]
            elif name.startswith("W"):
                shape = [C, C]
            else:
                shape = [C]
            ins[name] = nc.dram_tensor(name, shape, F32, kind="ExternalInput").ap()
        out = nc.dram_tensor("out", [T, C], F32, kind="ExternalOutput").ap()
        with tile.TileContext(nc) as tc:
            build_attn_tile(tc, {"out": out}, ins, T=T, C=C)
        nc.compile()
        _PROGRAM = nc
    return _PROGRAM


def _ensure_axon_hooks_stub():
    """bass_utils' trace path imports antenv.axon_hooks, which this image
    lacks; install a no-op registry so a stray BASS_TRACE env var degrades
    to 'no profile' instead of crashing."""
    import types

    try:
        import antenv.axon_hooks  # noqa: F401
        return
    except ImportError:
        pass
    import antenv

    mod = types.ModuleType("antenv.axon_hooks")
    _state = {"hook": None}
    mod.set_axon_ntff_profile_hook = lambda h: _state.__setitem__("hook", h)
    mod.get_axon_ntff_profile_hook = lambda: _state["hook"]
    sys.modules["antenv.axon_hooks"] = mod
    antenv.axon_hooks = mod


def run_spmd(inputs, trace=False):
    """Run the 8-core SPMD program; returns (out [B,H,W,C] f32, results obj)."""
    from concourse.bass_utils import run_bass_kernel_spmd

    _ensure_axon_hooks_stub()
    _install_neff_cache()
    nc = _get_program()
    arrs = {k: np.ascontiguousarray(np.asarray(v, dtype=np.float32)) for k, v in inputs.items()}
    x = arrs["x"].reshape(B, T, C)
    in_maps = []
    for b in range(B):
        m = {k: arrs[k] for k in IN_NAMES if k != "x"}
        m["x"] = x[b]
        in_maps.append(m)
    res = run_bass_kernel_spmd(nc, in_maps, core_ids=list(range(N_CORES)), trace=trace)
    out = np.stack([res.results[b]["out"] for b in range(B)], axis=0)
    return out.reshape(B, H, W, C).astype(np.float32), res


def kernel(**inputs) -> np.ndarray:
    out, _ = run_spmd(inputs, trace=False)
    return out
